# revision 1
# baseline (speedup 1.0000x reference)
"""Causal self-attention (B=2, T=2048, D=1024, 16 heads) on 8 TRN2 NeuronCores.

Sharding: 2 groups of 4 cores (one per batch element). Within a group each
core owns 4 heads for QKV+attention, and a 256-wide slice of the output
features for the final projection (after a 4-core AllGather of the per-head
attention outputs).

Per-core device program (identical SPMD program; per-core behavior comes
only from the input data):
  Phase 1: QKV projection from host-transposed xT [1024, 2048].
           Q,K produced feature-major ([64*heads, tok]); V token-major
           ([tok, 64*heads]) with an interleaved ones-column per head (the
           ones column makes the PV matmul also produce the softmax
           denominator for free).
  Phase 2: causal attention in "ST orientation": scores[tk, tq] tiles via
           PE (contraction over head dim, 2 heads packed in the partition
           dim -> concurrent row-group matmuls), exp on ACT into fp16 P
           tiles, causal mask via GPSIMD affine_select on diagonal tiles
           only, PV matmul accumulates [yT; den] in PSUM. Normalization via
           DVE reciprocal_approx_fast + GPSIMD partition_broadcast.
           No max-subtraction: scores are ~N(0, 0.33) after the 1/8 scale,
           so exp() is in [~0.08, ~8] and cannot overflow.
  Phase 3: 4-core AllGather of yT [256, 2048] -> [1024, 2048], then the
           output projection restricted to this core's 256 output features,
           with bias added on the ACT PSUM->SBUF evacuation.

All dense matmuls run as float32r (1 cycle/row on the PE for free dim
>= 256, vs 4 cycles/row for strict fp32).
"""

import sys

if "/opt/trn_rl_repo" not in sys.path:
    sys.path.insert(0, "/opt/trn_rl_repo")

import numpy as np

import concourse.bacc as bacc
import concourse.mybir as mybir
import concourse.tile as tile
from concourse.bass_utils import run_bass_kernel_spmd

dt = mybir.dt
F32 = dt.float32
F32R = dt.float32r
F16 = dt.float16
AF = mybir.ActivationFunctionType
ALU = mybir.AluOpType

B, T, C = 2, 2048, 1024
H, DH = 16, 64
HPC = 4              # heads per core
NG = 4               # cores per group
QB = 512             # tq block size
KB = 128             # tk chunk size
NQB = T // QB        # 4 query blocks
NKC = T // KB        # 16 key chunks
NCC = C // 128       # 8 contraction chunks of the model dim

_CACHE = {}


def _build(collective=True):
    nc = bacc.Bacc(
        "TRN2",
        target_bir_lowering=False,
        debug=False,
        enable_asserts=True,
        num_devices=8,
    )
    xT = nc.dram_tensor("xT", [C, T], F16, kind="ExternalInput")
    wqk = nc.dram_tensor("wqk", [C, 512], F16, kind="ExternalInput")
    wv = nc.dram_tensor("wv", [C, 256], F16, kind="ExternalInput")
    bqk = nc.dram_tensor("bqk", [128, 4], F32, kind="ExternalInput")
    bv = nc.dram_tensor("bv", [128, 256], F32, kind="ExternalInput")
    wp = nc.dram_tensor("wp", [C, 256], F16, kind="ExternalInput")
    bp = nc.dram_tensor("bp", [128, 2], F32, kind="ExternalInput")
    outT = nc.dram_tensor("outT", [256, T], F32, kind="ExternalOutput")

    with tile.TileContext(nc) as tc:
        with (
            tc.tile_pool(name="consts", bufs=1) as cpool,
            tc.tile_pool(name="w", bufs=1) as wpool,
            tc.tile_pool(name="qk", bufs=1) as qkpool,
            tc.tile_pool(name="vaug", bufs=1) as vpool,
            tc.tile_pool(name="p", bufs=8) as ppool,
            tc.tile_pool(name="norm", bufs=4) as npool,
            tc.tile_pool(name="y", bufs=4) as ypool,
            tc.tile_pool(name="ps_big", bufs=2, space="PSUM") as psb,
            tc.tile_pool(name="ps_st", bufs=4, space="PSUM") as pst,
            tc.tile_pool(name="ps_y", bufs=2, space="PSUM") as psy,
            tc.tile_pool(name="dram", bufs=1, space="DRAM") as dpool,
        ):
            # ---- constant / weight loads ----
            bqk_sb = cpool.tile([128, 4], F32, name="bqk_sb")
            bv_sb = cpool.tile([128, 256], F32, name="bv_sb")
            bp_sb = cpool.tile([128, 2], F32, name="bp_sb")
            nc.sync.dma_start(out=bqk_sb[:], in_=bqk[:, :])
            nc.sync.dma_start(out=bv_sb[:], in_=bv[:, :])
            nc.sync.dma_start(out=bp_sb[:], in_=bp[:, :])

            wqk_all = wpool.tile([128, NCC, 512], F16, tag="wqk", name="wqk_all")
            nc.scalar.dma_start(
                out=wqk_all[:], in_=wqk[:, :].rearrange("(a p) c -> p a c", p=128)
            )
            wqk_sb = [wqk_all[:, kc, :] for kc in range(NCC)]
            wv_all = wpool.tile([128, NCC, 256], F16, tag="wv", name="wv_all")
            nc.gpsimd.dma_start(
                out=wv_all[:], in_=wv[:, :].rearrange("(a p) c -> p a c", p=128)
            )
            wv_sb = [wv_all[:, kc, :] for kc in range(NCC)]

            # qk tiles: 0,1 = Q head-pairs (t=0: heads 0,1; t=1: heads 2,3)
            #           2,3 = K head-pairs
            qk_sb = [
                [
                    qkpool.tile([128, QB], F32R, tag=f"qk{i}_{j}", name=f"qk{i}_{j}")
                    for j in range(NQB)
                ]
                for i in range(4)
            ]
            vaug_sb = [vpool.tile([128, 260], F16, tag=f"va{i}", name=f"va{i}") for i in range(NKC)]

            ag_in = [
                dpool.tile([256, QB], F16, name=f"ag_in{j}", tag=f"ag_in{j}")
                for j in range(NQB)
            ]
            ag_out = [
                dpool.tile([C, QB], F16, name=f"ag_out{j}", tag=f"ag_out{j}")
                for j in range(NQB)
            ]

            # ---- phases 1+2 interleaved per token block (causal!) ----
            with tc.tile_pool(name="xt", bufs=1) as xtpool:
                xt_sb = [[None] * NQB for _ in range(NCC)]
                for tb in range(NQB):
                    for kc in range(NCC):
                        t_x = xtpool.tile(
                            [128, QB], F16, tag=f"xt{kc}_{tb}", name=f"xt{kc}_{tb}"
                        )
                        nc.sync.dma_start(
                            out=t_x[:],
                            in_=xT[
                                128 * kc : 128 * (kc + 1), QB * tb : QB * (tb + 1)
                            ],
                        )
                        xt_sb[kc][tb] = t_x

                for tb in range(NQB):
                    # Q,K feature-major for this token block
                    for f in range(4):
                        ps = psb.tile([128, 512], F32, tag="big", name="ps")
                        for kc in range(NCC):
                            nc.tensor.matmul(
                                ps[:],
                                wqk_sb[kc][:, 128 * f : 128 * (f + 1)],
                                xt_sb[kc][tb][:],
                                start=(kc == 0),
                                stop=(kc == NCC - 1),
                            )
                        nc.vector.tensor_scalar_add(
                            qk_sb[f][tb][:],
                            ps[:],
                            bqk_sb[:, f : f + 1],
                        )
                    # V token-major with ones columns for this token block
                    for tkc in range(4 * tb, 4 * tb + 4):
                        ps = psb.tile([128, 256], F32, tag="big", name="ps")
                        for kc in range(NCC):
                            nc.tensor.matmul(
                                ps[:],
                                xt_sb[kc][tkc // 4][:, KB * (tkc % 4) : KB * (tkc % 4 + 1)],
                                wv_sb[kc],
                                start=(kc == 0),
                                stop=(kc == NCC - 1),
                            )
                        va = vaug_sb[tkc]
                        va3 = va[:].rearrange("p (h z) -> p h z", z=65)
                        nc.vector.tensor_tensor(
                            out=va3[:, :, 0:64],
                            in0=ps[:].rearrange("p (h d) -> p h d", d=64),
                            in1=bv_sb[:].rearrange("p (h d) -> p h d", d=64),
                            op=ALU.add,
                        )
                        nc.vector.memset(va3[:, :, 64:65], 1.0)

                    tqb = tb
                    nchunks = 4 * tqb + 4
                    for t in range(2):
                        y_ps = [
                            psy.tile([65, 512], F32, tag="y", name="y_ps")
                            for _ in range(2)
                        ]
                        for i in range(nchunks):
                            d = i - 4 * tqb
                            co = 128 * d if d > 0 else 0  # fully-masked cols skipped
                            for s in range(2):
                                h = 2 * t + s
                                st = pst.tile([128, 512], F32, tag="st", name="st")
                                nc.tensor.matmul(
                                    st[:, co:512],
                                    qk_sb[2 + t][i // 4][
                                        64 * s : 64 * (s + 1),
                                        KB * (i % 4) : KB * (i % 4 + 1),
                                    ],
                                    qk_sb[t][tqb][64 * s : 64 * (s + 1), co:512],
                                    start=True,
                                    stop=True,
                                )
                                p = ppool.tile([128, 512], F16, tag="p", name="p")
                                nc.scalar.activation(
                                    p[:, co:512], st[:, co:512], AF.Exp, scale=0.125
                                )
                                if d >= 0:
                                    nc.gpsimd.affine_select(
                                        out=p[:, co : co + 128],
                                        in_=p[:, co : co + 128],
                                        compare_op=ALU.is_ge,
                                        fill=0.0,
                                        base=0,
                                        pattern=[[1, 128]],
                                        channel_multiplier=-1,
                                    )
                                nc.tensor.matmul(
                                    y_ps[s][:, co:512],
                                    vaug_sb[i][:, 65 * h : 65 * h + 65],
                                    p[:, co:512],
                                    start=(i == 0),
                                    stop=(i == nchunks - 1),
                                )
                        for s in range(2):
                            h = 2 * t + s
                            yp = y_ps[s]
                            den_hi = npool.tile(
                                [65, 512], F32, tag="den_hi", name="den_hi"
                            )
                            nc.vector.tensor_copy(den_hi[64:65, :], yp[64:65, :])
                            den0 = npool.tile([1, 512], F32, tag="den0", name="den0")
                            nc.sync.dma_start(out=den0[:], in_=den_hi[64:65, :])
                            recip = npool.tile([1, 512], F32, tag="recip", name="recip")
                            nc.vector.reciprocal_approx_fast(out=recip[:], in_=den0[:])
                            rb = npool.tile([64, 512], F32, tag="rb", name="rb")
                            nc.gpsimd.partition_broadcast(rb[:], recip[:])
                            ysb = ypool.tile([64, 512], F16, tag="ysb", name="ysb")
                            nc.vector.tensor_tensor(
                                out=ysb[:], in0=yp[0:64, :], in1=rb[:], op=ALU.mult
                            )
                            nc.sync.dma_start(
                                out=ag_in[tqb][64 * h : 64 * (h + 1), :],
                                in_=ysb[:],
                            )
                    if collective:
                        nc.gpsimd.collective_compute(
                            "AllGather",
                            ALU.bypass,
                            replica_groups=[[0, 1, 2, 3], [4, 5, 6, 7]],
                            ins=[ag_in[tb][:]],
                            outs=[ag_out[tb][:]],
                        )
                    else:
                        # timing-only stand-in (TimelineSim has no collectives)
                        nc.sync.dma_start(
                            out=ag_out[tb][0:256, :], in_=ag_in[tb][:]
                        )

            # ---- phase 3: output projection ----
            with tc.tile_pool(name="yf", bufs=2) as yfpool:
                wp_all = yfpool.tile(
                    [128, NCC, 256], F16, tag="wp", name="wp_all", bufs=1
                )
                nc.gpsimd.dma_start(
                    out=wp_all[:], in_=wp[:, :].rearrange("(a p) c -> p a c", p=128)
                )
                wp_sb = [wp_all[:, kc, :] for kc in range(NCC)]

                for tb in range(NQB):
                    yf_sb = []
                    for half in range(2):
                        t_y = yfpool.tile(
                            [128, 4, QB], F16, tag=f"yfh{half}", name=f"yfh{half}"
                        )
                        nc.sync.dma_start(
                            out=t_y[:],
                            in_=ag_out[tb][
                                512 * half : 512 * (half + 1), :
                            ].rearrange("(a p) c -> p a c", p=128),
                        )
                        yf_sb.extend(t_y[:, j, :] for j in range(4))
                    for m in range(2):
                        ps = psb.tile([128, 512], F32, tag="big", name="ps")
                        for kc in range(NCC):
                            nc.tensor.matmul(
                                ps[:],
                                wp_sb[kc][:, 128 * m : 128 * (m + 1)],
                                yf_sb[kc],
                                start=(kc == 0),
                                stop=(kc == NCC - 1),
                            )
                        osb = ypool.tile([128, 512], F32, tag="osb", name="osb")
                        nc.scalar.activation(
                            osb[:], ps[:], AF.Identity, bias=bp_sb[:, m : m + 1]
                        )
                        nc.sync.dma_start(
                            out=outT[128 * m : 128 * (m + 1), QB * tb : QB * (tb + 1)],
                            in_=osb[:],
                        )
    nc.finalize()
    return nc


def _get_nc():
    if "nc" not in _CACHE:
        _CACHE["nc"] = _build()
    return _CACHE["nc"]


def _make_in_maps(x, W_attn, b_attn, W_proj, b_proj):
    x = np.asarray(x, dtype=np.float32)
    W_attn = np.asarray(W_attn, dtype=np.float32)
    b_attn = np.asarray(b_attn, dtype=np.float32)
    W_proj = np.asarray(W_proj, dtype=np.float32)
    b_proj = np.asarray(b_proj, dtype=np.float32)

    xTs = [np.ascontiguousarray(x[g].T.astype(np.float16)) for g in range(B)]
    in_maps = []
    for c in range(8):
        g, r = c // NG, c % NG
        h0 = HPC * r
        q0, k0, v0 = 64 * h0, C + 64 * h0, 2 * C + 64 * h0
        wqk = np.ascontiguousarray(
            np.concatenate(
                [W_attn[:, q0 : q0 + 256], W_attn[:, k0 : k0 + 256]], axis=1
            ).astype(np.float16)
        )
        wv = np.ascontiguousarray(W_attn[:, v0 : v0 + 256].astype(np.float16))
        bqk = np.ascontiguousarray(
            np.concatenate(
                [b_attn[q0 : q0 + 256], b_attn[k0 : k0 + 256]]
            ).reshape(4, 128).T
        )
        bv = np.ascontiguousarray(
            np.broadcast_to(b_attn[v0 : v0 + 256], (128, 256))
        )
        wp = np.ascontiguousarray(W_proj[:, 256 * r : 256 * (r + 1)].astype(np.float16))
        bp = np.ascontiguousarray(
            b_proj[256 * r : 256 * (r + 1)].reshape(2, 128).T
        )
        in_maps.append(
            {
                "xT": xTs[g],
                "wqk": wqk,
                "wv": wv,
                "bqk": bqk,
                "bv": bv,
                "wp": wp,
                "bp": bp,
            }
        )
    return in_maps


def _assemble(results):
    out = np.empty((B, T, C), dtype=np.float32)
    for c in range(8):
        g, r = c // NG, c % NG
        out[g, :, 256 * r : 256 * (r + 1)] = results[c]["outT"].T
    return out


def kernel(x, W_attn, b_attn, W_proj, b_proj, _trace=False):
    import time

    nc = _get_nc()
    in_maps = _make_in_maps(x, W_attn, b_attn, W_proj, b_proj)
    last_err = None
    for attempt in range(3):
        try:
            res = run_bass_kernel_spmd(nc, in_maps, list(range(8)), trace=_trace)
            break
        except Exception as e:  # device occasionally wedges on first exec
            last_err = e
            time.sleep(20 * (attempt + 1))
    else:
        raise last_err
    out = _assemble(res.results)
    if _trace:
        return out, res
    return out



# revision 13
# speedup vs baseline: 1.0400x; 1.0400x over previous
"""Causal self-attention (B=2, T=2048, D=1024, 16 heads) on 8 TRN2 NeuronCores.

Sharding: 2 groups of 4 cores (one per batch element). Within a group each
core owns 4 heads for QKV+attention, and a 256-wide slice of the output
features for the final projection (after a 4-core AllGather of the per-head
attention outputs).

Per-core device program (identical SPMD program; per-core behavior comes
only from the input data):
  Phase 1: QKV projection from host-transposed xT [1024, 2048].
           Q,K produced feature-major ([64*heads, tok]); V token-major
           ([tok, 64*heads]) with an interleaved ones-column per head (the
           ones column makes the PV matmul also produce the softmax
           denominator for free).
  Phase 2: causal attention in "ST orientation": scores[tk, tq] tiles via
           PE (contraction over head dim, 2 heads packed in the partition
           dim -> concurrent row-group matmuls), exp on ACT into fp16 P
           tiles, causal mask via GPSIMD affine_select on diagonal tiles
           only, PV matmul accumulates [yT; den] in PSUM. Normalization via
           DVE reciprocal_approx_fast + GPSIMD partition_broadcast.
           No max-subtraction: scores are ~N(0, 0.33) after the 1/8 scale,
           so exp() is in [~0.08, ~8] and cannot overflow.
  Phase 3: 4-core AllGather of yT [256, 2048] -> [1024, 2048], then the
           output projection restricted to this core's 256 output features,
           with bias added on the ACT PSUM->SBUF evacuation.

All dense matmuls run as float32r (1 cycle/row on the PE for free dim
>= 256, vs 4 cycles/row for strict fp32).
"""

import sys

if "/opt/trn_rl_repo" not in sys.path:
    sys.path.insert(0, "/opt/trn_rl_repo")

import numpy as np

import concourse.bacc as bacc
import concourse.mybir as mybir
import concourse.tile as tile
from concourse.bass_utils import run_bass_kernel_spmd

dt = mybir.dt
F32 = dt.float32
F32R = dt.float32r
F16 = dt.float16
AF = mybir.ActivationFunctionType
ALU = mybir.AluOpType

B, T, C = 2, 2048, 1024
H, DH = 16, 64
HPC = 4              # heads per core
NG = 4               # cores per group
QB = 512             # tq block size
KB = 128             # tk chunk size
NQB = T // QB        # 4 query blocks
NKC = T // KB        # 16 key chunks
NCC = C // 128       # 8 contraction chunks of the model dim

_CACHE = {}


def _build(collective=True):
    nc = bacc.Bacc(
        "TRN2",
        target_bir_lowering=False,
        debug=False,
        enable_asserts=True,
        num_devices=8,
    )
    xT = nc.dram_tensor("xT", [C, T], F16, kind="ExternalInput")
    wqk = nc.dram_tensor("wqk", [C, 512], F16, kind="ExternalInput")
    wv = nc.dram_tensor("wv", [C, 256], F16, kind="ExternalInput")
    bqk = nc.dram_tensor("bqk", [128, 4], F32, kind="ExternalInput")
    bv = nc.dram_tensor("bv", [128, 256], F32, kind="ExternalInput")
    wp = nc.dram_tensor("wp", [C, 256], F16, kind="ExternalInput")
    bp = nc.dram_tensor("bp", [128, 2], F32, kind="ExternalInput")
    outT = nc.dram_tensor("outT", [256, T], F32, kind="ExternalOutput")

    with tile.TileContext(nc) as tc:
        with (
            tc.tile_pool(name="consts", bufs=1) as cpool,
            tc.tile_pool(name="w", bufs=1) as wpool,
            tc.tile_pool(name="qk", bufs=1) as qkpool,
            tc.tile_pool(name="vaug", bufs=1) as vpool,
            tc.tile_pool(name="p", bufs=8) as ppool,
            tc.tile_pool(name="norm", bufs=4) as npool,
            tc.tile_pool(name="y", bufs=4) as ypool,
            tc.tile_pool(name="ps_big", bufs=2, space="PSUM") as psb,
            tc.tile_pool(name="ps_st", bufs=4, space="PSUM") as pst,
            tc.tile_pool(name="ps_y", bufs=2, space="PSUM") as psy,
            tc.tile_pool(name="dram", bufs=1, space="DRAM") as dpool,
        ):
            # ---- constant / weight loads ----
            bqk_sb = cpool.tile([128, 4], F32, name="bqk_sb")
            bv_sb = cpool.tile([128, 256], F32, name="bv_sb")
            bp_sb = cpool.tile([128, 2], F32, name="bp_sb")
            nc.sync.dma_start(out=bqk_sb[:], in_=bqk[:, :])
            nc.sync.dma_start(out=bv_sb[:], in_=bv[:, :])
            nc.sync.dma_start(out=bp_sb[:], in_=bp[:, :])

            wqk_t = [
                wpool.tile([128, 512], F16, tag=f"wqk{kc}", name=f"wqk{kc}")
                for kc in range(NCC)
            ]
            wqkr = wqk[:, :].rearrange("(a p) c -> p a c", p=128)
            for kc in range(NCC):
                nc.scalar.dma_start(out=wqk_t[kc][:], in_=wqkr[:, kc, :])
            wqk_sb = [wqk_t[kc][:] for kc in range(NCC)]
            wv_all = wpool.tile([128, NCC, 256], F16, tag="wv", name="wv_all")
            nc.gpsimd.dma_start(
                out=wv_all[:], in_=wv[:, :].rearrange("(a p) c -> p a c", p=128)
            )
            wv_sb = [wv_all[:, kc, :] for kc in range(NCC)]

            # qk tiles: 0,1 = Q head-pairs (t=0: heads 0,1; t=1: heads 2,3)
            #           2,3 = K head-pairs
            qk_sb = [
                [
                    qkpool.tile([128, QB], F32R, tag=f"qk{i}_{j}", name=f"qk{i}_{j}")
                    for j in range(NQB)
                ]
                for i in range(4)
            ]
            vaug_sb = [vpool.tile([128, 260], F16, tag=f"va{i}", name=f"va{i}") for i in range(NKC)]

            ag_in = [
                dpool.tile([256, QB], F16, name=f"ag_in{j}", tag=f"ag_in{j}")
                for j in range(NQB)
            ]
            ag_out = [
                dpool.tile([C, QB], F16, name=f"ag_out{j}", tag=f"ag_out{j}")
                for j in range(NQB)
            ]

            # ---- phases 1+2 interleaved per token block (causal!) ----
            with tc.tile_pool(name="xt", bufs=1) as xtpool:
                xt_sb = [[None] * NQB for _ in range(NCC)]
                for tb in range(NQB):
                    for kc in range(NCC):
                        t_x = xtpool.tile(
                            [128, QB], F16, tag=f"xt{kc}_{tb}", name=f"xt{kc}_{tb}"
                        )
                        nc.sync.dma_start(
                            out=t_x[:],
                            in_=xT[
                                128 * kc : 128 * (kc + 1), QB * tb : QB * (tb + 1)
                            ],
                        )
                        xt_sb[kc][tb] = t_x

                for tb in range(NQB):
                    # Q,K feature-major for this token block
                    for f in range(4):
                        ps = psb.tile([128, 512], F32, tag="big", name="ps")
                        for kc in range(NCC):
                            nc.tensor.matmul(
                                ps[:],
                                wqk_sb[kc][:, 128 * f : 128 * (f + 1)],
                                xt_sb[kc][tb][:],
                                start=(kc == 0),
                                stop=(kc == NCC - 1),
                            )
                        nc.vector.tensor_scalar_add(
                            qk_sb[f][tb][:],
                            ps[:],
                            bqk_sb[:, f : f + 1],
                        )
                    # V token-major with ones columns for this token block
                    for tkc in range(4 * tb, 4 * tb + 4):
                        ps = psb.tile([128, 256], F32, tag="big", name="ps")
                        for kc in range(NCC):
                            nc.tensor.matmul(
                                ps[:],
                                xt_sb[kc][tkc // 4][:, KB * (tkc % 4) : KB * (tkc % 4 + 1)],
                                wv_sb[kc],
                                start=(kc == 0),
                                stop=(kc == NCC - 1),
                            )
                        va = vaug_sb[tkc]
                        va3 = va[:].rearrange("p (h z) -> p h z", z=65)
                        nc.vector.tensor_tensor(
                            out=va3[:, :, 0:64],
                            in0=ps[:].rearrange("p (h d) -> p h d", d=64),
                            in1=bv_sb[:].rearrange("p (h d) -> p h d", d=64),
                            op=ALU.add,
                        )
                        nc.vector.memset(va3[:, :, 64:65], 1.0)

                    tqb = tb
                    nchunks = 4 * tqb + 4
                    for t in range(2):
                        y_ps = [
                            psy.tile([65, 512], F32, tag="y", name="y_ps")
                            for _ in range(2)
                        ]
                        for i in range(nchunks):
                            d = i - 4 * tqb
                            co = 128 * d if d > 0 else 0  # fully-masked cols skipped
                            for s in range(2):
                                h = 2 * t + s
                                st = pst.tile([128, 512], F32, tag="st", name="st")
                                nc.tensor.matmul(
                                    st[:, co:512],
                                    qk_sb[2 + t][i // 4][
                                        64 * s : 64 * (s + 1),
                                        KB * (i % 4) : KB * (i % 4 + 1),
                                    ],
                                    qk_sb[t][tqb][64 * s : 64 * (s + 1), co:512],
                                    start=True,
                                    stop=True,
                                )
                                p = ppool.tile([128, 512], F16, tag="p", name="p")
                                nc.scalar.activation(
                                    p[:, co:512], st[:, co:512], AF.Exp, scale=0.125
                                )
                                if d >= 0:
                                    nc.gpsimd.affine_select(
                                        out=p[:, co : co + 128],
                                        in_=p[:, co : co + 128],
                                        compare_op=ALU.is_ge,
                                        fill=0.0,
                                        base=0,
                                        pattern=[[1, 128]],
                                        channel_multiplier=-1,
                                    )
                                nc.tensor.matmul(
                                    y_ps[s][:, co:512],
                                    vaug_sb[i][:, 65 * h : 65 * h + 65],
                                    p[:, co:512],
                                    start=(i == 0),
                                    stop=(i == nchunks - 1),
                                )
                        for s in range(2):
                            h = 2 * t + s
                            yp = y_ps[s]
                            den_hi = npool.tile(
                                [65, 512], F32, tag="den_hi", name="den_hi"
                            )
                            nc.vector.tensor_copy(den_hi[64:65, :], yp[64:65, :])
                            den0 = npool.tile([1, 512], F32, tag="den0", name="den0")
                            nc.sync.dma_start(out=den0[:], in_=den_hi[64:65, :])
                            recip = npool.tile([1, 512], F32, tag="recip", name="recip")
                            nc.vector.reciprocal_approx_fast(out=recip[:], in_=den0[:])
                            rb = npool.tile([64, 512], F32, tag="rb", name="rb")
                            nc.gpsimd.partition_broadcast(rb[:], recip[:])
                            ysb = ypool.tile([64, 512], F16, tag="ysb", name="ysb")
                            nc.vector.tensor_tensor(
                                out=ysb[:], in0=yp[0:64, :], in1=rb[:], op=ALU.mult
                            )
                            nc.sync.dma_start(
                                out=ag_in[tqb][64 * h : 64 * (h + 1), :],
                                in_=ysb[:],
                            )
                    if collective:
                        nc.gpsimd.collective_compute(
                            "AllGather",
                            ALU.bypass,
                            replica_groups=[[0, 1, 2, 3], [4, 5, 6, 7]],
                            ins=[ag_in[tb][:]],
                            outs=[ag_out[tb][:]],
                        )
                    else:
                        # timing-only stand-in (TimelineSim has no collectives)
                        nc.sync.dma_start(
                            out=ag_out[tb][0:256, :], in_=ag_in[tb][:]
                        )

            # ---- phase 3: output projection ----
            with tc.tile_pool(name="yf", bufs=2) as yfpool:
                wp_all = yfpool.tile(
                    [128, NCC, 256], F16, tag="wp", name="wp_all", bufs=1
                )
                nc.gpsimd.dma_start(
                    out=wp_all[:], in_=wp[:, :].rearrange("(a p) c -> p a c", p=128)
                )
                wp_sb = [wp_all[:, kc, :] for kc in range(NCC)]

                for tb in range(NQB):
                    if tb == NQB - 1:
                        for w in range(10):
                            ps = psb.tile([128, 512], F32, tag="big", name="ps")
                            for kc in range(NCC):
                                nc.tensor.matmul(
                                    ps[:, 0:256],
                                    wp_sb[kc][:, 0:128],
                                    wp_sb[kc],
                                    start=(kc == 0),
                                    stop=(kc == NCC - 1),
                                )
                    yf_sb = []
                    for half in range(2):
                        t_y = yfpool.tile(
                            [128, 4, QB], F16, tag=f"yfh{half}", name=f"yfh{half}"
                        )
                        nc.sync.dma_start(
                            out=t_y[:],
                            in_=ag_out[tb][
                                512 * half : 512 * (half + 1), :
                            ].rearrange("(a p) c -> p a c", p=128),
                        )
                        yf_sb.extend(t_y[:, j, :] for j in range(4))
                    for m in range(2):
                        ps = psb.tile([128, 512], F32, tag="big", name="ps")
                        for kc in range(NCC):
                            nc.tensor.matmul(
                                ps[:],
                                wp_sb[kc][:, 128 * m : 128 * (m + 1)],
                                yf_sb[kc],
                                start=(kc == 0),
                                stop=(kc == NCC - 1),
                            )
                        osb = ypool.tile([128, 512], F32, tag="osb", name="osb")
                        nc.scalar.activation(
                            osb[:], ps[:], AF.Identity, bias=bp_sb[:, m : m + 1]
                        )
                        nc.sync.dma_start(
                            out=outT[128 * m : 128 * (m + 1), QB * tb : QB * (tb + 1)],
                            in_=osb[:],
                        )
    nc.finalize()
    return nc


def _get_nc():
    if "nc" not in _CACHE:
        _CACHE["nc"] = _build()
    return _CACHE["nc"]


def _make_in_maps(x, W_attn, b_attn, W_proj, b_proj):
    x = np.asarray(x, dtype=np.float32)
    W_attn = np.asarray(W_attn, dtype=np.float32)
    b_attn = np.asarray(b_attn, dtype=np.float32)
    W_proj = np.asarray(W_proj, dtype=np.float32)
    b_proj = np.asarray(b_proj, dtype=np.float32)

    xTs = [np.ascontiguousarray(x[g].T.astype(np.float16)) for g in range(B)]
    in_maps = []
    for c in range(8):
        g, r = c // NG, c % NG
        h0 = HPC * r
        q0, k0, v0 = 64 * h0, C + 64 * h0, 2 * C + 64 * h0
        wqk = np.ascontiguousarray(
            np.concatenate(
                [W_attn[:, q0 : q0 + 256], W_attn[:, k0 : k0 + 256]], axis=1
            ).astype(np.float16)
        )
        wv = np.ascontiguousarray(W_attn[:, v0 : v0 + 256].astype(np.float16))
        bqk = np.ascontiguousarray(
            np.concatenate(
                [b_attn[q0 : q0 + 256], b_attn[k0 : k0 + 256]]
            ).reshape(4, 128).T
        )
        bv = np.ascontiguousarray(
            np.broadcast_to(b_attn[v0 : v0 + 256], (128, 256))
        )
        wp = np.ascontiguousarray(W_proj[:, 256 * r : 256 * (r + 1)].astype(np.float16))
        bp = np.ascontiguousarray(
            b_proj[256 * r : 256 * (r + 1)].reshape(2, 128).T
        )
        in_maps.append(
            {
                "xT": xTs[g],
                "wqk": wqk,
                "wv": wv,
                "bqk": bqk,
                "bv": bv,
                "wp": wp,
                "bp": bp,
            }
        )
    return in_maps


def _assemble(results):
    out = np.empty((B, T, C), dtype=np.float32)
    for c in range(8):
        g, r = c // NG, c % NG
        out[g, :, 256 * r : 256 * (r + 1)] = results[c]["outT"].T
    return out


def kernel(x, W_attn, b_attn, W_proj, b_proj, _trace=False):
    import time

    nc = _get_nc()
    in_maps = _make_in_maps(x, W_attn, b_attn, W_proj, b_proj)
    last_err = None
    for attempt in range(3):
        try:
            res = run_bass_kernel_spmd(nc, in_maps, list(range(8)), trace=_trace)
            break
        except Exception as e:  # device occasionally wedges on first exec
            last_err = e
            time.sleep(20 * (attempt + 1))
    else:
        raise last_err
    out = _assemble(res.results)
    if _trace:
        return out, res
    return out

